# revision 1
# baseline (speedup 1.0000x reference)
"""VQ codebook soft-assignment encoding kernel for 8 trn2 NeuronCores.

Math (per batch b):
  Xf = X[b].reshape(D, N).T                        # [N, D], N = H*W
  logit[n,k] = scale[k] * (||x_n||^2 - 2<x_n,c_k> + ||c_k||^2)
  A = softmax(logit, axis=k)
  E[b,k,:] = sum_n A[n,k] * (x_n - c_k)            # [K, D]

Sharding: data-parallel over B (4 batches per core), codewords/scale replicated.

Device plan per core (all heavy compute in fp16 with fp32 PSUM accumulation):
  - SWDGE cast-load X[b] fp32 HBM -> fp16 SBUF, native [d, n] layout
  - logits in [k, n] layout:  PSUM = G^T X + S^T X^2  where G[d,k] = -2 s_k c[k,d],
    S[d,k] = s_k  (so S^T X^2 contributes s_k*||x_n||^2);  then
    U = exp(PSUM + bias_k),  bias_k = s_k ||c_k||^2  (ACT, per-partition bias)
  - DMA-xbar transposes (fp16): X -> XT [128, 72, 264] tiles ([n-part, d], col 256 = ones),
    U -> UT [128, 72, 32]
  - normalize on DVE: den = reduce_k UT, A = UT * (1/den)
  - E-matmul on PE: psE[32, 257] += A_t^T @ XT_t over 72 n-chunks
    -> cols 0:256 = sum_n A[n,k] x[n,d], col 256 = S_k = sum_n A[n,k]
  - E = psE[:, :256] - S_k * c  (DVE), DMA out fp32
"""
import numpy as np
from contextlib import ExitStack

import concourse.bass as bass
import concourse.mybir as mybir
import concourse.tile as tile
from concourse.tile import ScopedClock
from concourse.bass_utils import run_bass_kernel_spmd

dt = mybir.dt

B, D, K, H, W = 32, 256, 32, 96, 96
N = H * W                 # 9216
NCORES = 8
BPC = B // NCORES         # 4 batches per core
TN = 512                  # n-tile for logits pass
NT = N // TN              # 18
NCHUNK = N // 128         # 72 e-matmul chunks
SQG = 3                   # n-tiles per square group
XTW = 264                 # XT row width (256 d + 1 ones + pad)


def _patch_tile_drain():
    """This toolchain's walrus allows only one sync-wait per instruction.
    Split the tail drain's waits across chained drains."""
    if getattr(tile.TileContext, "_drain_patched", False):
        return

    def _drain_and_barrier_split(self, tick_clock, wait_clock):
        nc = self.nc
        drain_inst = nc.sync.drain()
        wait_clock.add_sem_waits(
            drain_inst.ins, ScopedClock({None: tick_clock.global_clock})
        )
        si = drain_inst.ins.sync_info
        if si is not None and si.on_wait and len(si.on_wait) > 1:
            extra = list(si.on_wait[1:])
            del si.on_wait[1:]
            for w in extra:
                d = nc.sync.drain()
                dsi = d.ins.sync_info
                if dsi is None:
                    d.ins.sync_info = mybir.SyncInfo(on_wait=[w], on_update=[])
                else:
                    dsi.on_wait.append(w)
        nc.all_engine_barrier()
        assert self.sems is not None
        popped = nc._tile_sem_poison_stack.pop()
        assert popped is self._sem_poison
        nc.clear_and_free_semaphores(list(self.sems.allocated().values()))
        nc.all_engine_barrier()

    tile.TileContext._drain_and_barrier = _drain_and_barrier_split
    tile.TileContext._drain_patched = True


def _split_multi_waits(nc):
    """Hoist extra sem-waits onto standalone event-sem instructions."""
    n_split = 0
    for f in nc.m.functions:
        for bb in f.blocks:
            new_list = []
            for inst in bb.instructions:
                si = inst.sync_info
                if si is not None and si.on_wait is not None and len(si.on_wait) > 1:
                    extra = list(si.on_wait[:-1])
                    keep = [si.on_wait[-1]]
                    for w in extra:
                        ev = mybir.InstEventSemaphore(
                            name=f"{inst.name}-wsplit{n_split}",
                            ins=[], outs=[],
                            sync_info=mybir.SyncInfo(on_wait=[w], on_update=[]),
                        )
                        ev.engine = inst.engine
                        nc.register_instruction(ev)
                        new_list.append(ev)
                        n_split += 1
                    del si.on_wait[:]
                    si.on_wait.extend(keep)
                new_list.append(inst)
            bb.instructions[:] = new_list
    return n_split


def _build_module():
    _patch_tile_drain()
    nc = bass.Bass()
    xin = nc.declare_dram_parameter("xin", [BPC, D, N], dt.float32, isOutput=False)
    cw = nc.declare_dram_parameter("cw", [K, D], dt.float32, isOutput=False)
    s_col = nc.declare_dram_parameter("s_col", [K, 1], dt.float32, isOutput=False)
    s_row = nc.declare_dram_parameter("s_row", [1, K], dt.float32, isOutput=False)
    eout = nc.declare_dram_parameter("eout", [BPC, K, D], dt.float32, isOutput=True)

    f16, f32 = dt.float16, dt.float32
    AX = mybir.AxisListType.X
    EXP = mybir.ActivationFunctionType.Exp
    MUL = mybir.AluOpType.mult

    with tile.TileContext(nc) as tc:
        with ExitStack() as ctx:
            singles = ctx.enter_context(tc.tile_pool(name="singles", bufs=1))
            psprep = ctx.enter_context(tc.tile_pool(name="psprep", bufs=1, space="PSUM"))

            # ---- one-time prep from codewords/scale ----
            cw_sb = singles.tile([K, D], f32)
            nc.sync.dma_start(cw_sb[:], cw[:])
            scol_sb = singles.tile([K, 1], f32)
            nc.sync.dma_start(scol_sb[:], s_col[:])
            srow_sb = singles.tile([1, K], f32)
            nc.sync.dma_start(srow_sb[:], s_row[:])

            # G16 [128, 2, K]: G[p, c, k] = -2 s_k c[k, c*128+p]
            w1 = singles.tile([K, D], f32)
            nc.vector.tensor_scalar_mul(w1[:], cw_sb[:], scol_sb[:])
            w2 = singles.tile([K, D], f32)
            nc.vector.tensor_scalar_mul(w2[:], w1[:], -2.0)
            w16 = singles.tile([K, D], f16)
            nc.vector.tensor_copy(w16[:], w2[:])
            g16 = singles.tile([128, 2 * K], f16)
            nc.sync.dma_start_transpose(
                g16[:].rearrange("p (c k) -> p c k", k=K), w16[:]
            )

            # S16 [128, K]: every row = s_k (fp16)
            ones_row16 = singles.tile([1, 128], f16)
            nc.vector.memset(ones_row16[:], 1.0)
            srow16 = singles.tile([1, K], f16)
            nc.vector.tensor_copy(srow16[:], srow_sb[:])
            ps_s = psprep.tile([128, K], f32)
            nc.tensor.matmul(ps_s[:], ones_row16[:], srow16[:], start=True, stop=True,
                             skip_group_check=True)
            s16 = singles.tile([128, K], f16)
            nc.vector.tensor_copy(s16[:], ps_s[:])

            # bias [K, 1] = s_k * ||c_k||^2
            csq = singles.tile([K, D], f32)
            nc.vector.tensor_mul(csq[:], cw_sb[:], cw_sb[:])
            sqc = singles.tile([K, 1], f32)
            nc.vector.reduce_sum(
                sqc[:].rearrange("k (o p) -> k o p", o=1),
                csq[:].rearrange("k (o d) -> k o d", o=1), axis=AX)
            bias = singles.tile([K, 1], f32)
            nc.vector.tensor_mul(bias[:], sqc[:], scol_sb[:])

            ones_col16 = singles.tile([128, 1], f16)
            nc.vector.memset(ones_col16[:], 1.0)

            # ---- per-batch pools ----
            xpool = ctx.enter_context(tc.tile_pool(name="x16", bufs=2))
            xtpool = ctx.enter_context(tc.tile_pool(name="xt", bufs=2))
            sqpool = ctx.enter_context(tc.tile_pool(name="xsq", bufs=2))
            upool = ctx.enter_context(tc.tile_pool(name="u16", bufs=1))
            utpool = ctx.enter_context(tc.tile_pool(name="ut", bufs=1))
            apool = ctx.enter_context(tc.tile_pool(name="a16", bufs=2))
            npool = ctx.enter_context(tc.tile_pool(name="nrm", bufs=3))
            opool = ctx.enter_context(tc.tile_pool(name="out", bufs=2))
            psl = ctx.enter_context(tc.tile_pool(name="psl", bufs=3, space="PSUM"))
            pse = ctx.enter_context(tc.tile_pool(name="pse", bufs=2, space="PSUM"))

            for b in range(BPC):
                x0 = xpool.tile([128, N], f16, tag="x0")
                nc.gpsimd.dma_start(x0[:], xin[b, 0:128, :])
                x1 = xpool.tile([128, N], f16, tag="x1")
                nc.gpsimd.dma_start(x1[:], xin[b, 128:256, :])

                xt0 = xtpool.tile([128, NCHUNK * 128], f16, tag="xt0")
                xt0v = xt0[:].rearrange("p (t w) -> p t w", w=128)
                nc.sync.dma_start_transpose(xt0v, x0[:])
                xt1 = xtpool.tile([128, NCHUNK * 128], f16, tag="xt1")
                xt1v = xt1[:].rearrange("p (t w) -> p t w", w=128)
                nc.sync.dma_start_transpose(xt1v, x1[:])

                u16 = upool.tile([K, N], f16)
                for g in range(NT // SQG):
                    xsq = sqpool.tile([128, 2 * SQG * TN], f16, tag="xsq")
                    xsq3 = xsq[:].rearrange("p (c m) -> p c m", c=2)
                    sl = bass.ts(g, SQG * TN)
                    nc.vector.tensor_mul(xsq3[:, 0, :], x0[:, sl], x0[:, sl])
                    nc.vector.tensor_mul(xsq3[:, 1, :], x1[:, sl], x1[:, sl])
                    for j in range(SQG):
                        i = g * SQG + j
                        pl = psl.tile([K, TN], f32)
                        xs = bass.ts(i, TN)
                        js = bass.ts(j, TN)
                        nc.tensor.matmul(pl[:], g16[:].rearrange("p (c k) -> p c k", k=K)[:, 0, :],
                                         x0[:, xs], start=True, stop=False, skip_group_check=True)
                        nc.tensor.matmul(pl[:], g16[:].rearrange("p (c k) -> p c k", k=K)[:, 1, :],
                                         x1[:, xs], start=False, stop=False, skip_group_check=True)
                        nc.tensor.matmul(pl[:], s16[:], xsq3[:, 0, js],
                                         start=False, stop=False, skip_group_check=True)
                        nc.tensor.matmul(pl[:], s16[:], xsq3[:, 1, js],
                                         start=False, stop=True, skip_group_check=True)
                        nc.scalar.activation(u16[:, xs], pl[:], EXP, bias=bias[:], scale=1.0)

                ut = utpool.tile([128, NCHUNK * K], f16)
                ut3 = ut[:].rearrange("p (t k) -> p t k", k=K)
                nc.sync.dma_start_transpose(ut3, u16[:])

                a16 = apool.tile([128, NCHUNK * K], f16)
                a3 = a16[:].rearrange("p (t k) -> p t k", k=K)
                NG = 4
                for g in range(NCHUNK // NG):
                    den = npool.tile([128, NG], f32, tag="den")
                    den3 = den[:].rearrange("p (t o) -> p t o", o=1)
                    sl3 = ut3[:, g * NG:(g + 1) * NG, :]
                    nc.vector.reduce_sum(den3, sl3, axis=AX)
                    rec = npool.tile([128, NG], f32, tag="rec")
                    nc.vector.reciprocal(rec[:], den[:])
                    recb = rec[:].rearrange("p (t o) -> p t o", o=1).broadcast_to((128, NG, K))
                    nc.vector.tensor_mul(a3[:, g * NG:(g + 1) * NG, :], sl3, recb)

                pe = pse.tile([K, 257], f32)
                for t in range(NCHUNK):
                    sp = (t == NCHUNK - 1)
                    # start=True clears has_written for the whole PSUM bank, so
                    # only the very first matmul touching this bank may set it.
                    nc.tensor.matmul(pe[:, 0:128], a3[:, t, :], xt0v[:, t, :],
                                     start=(t == 0), stop=sp, skip_group_check=True)
                    nc.tensor.matmul(pe[:, 128:256], a3[:, t, :], xt1v[:, t, :],
                                     start=False, stop=sp, skip_group_check=True)
                    nc.tensor.matmul(pe[:, 256:257], a3[:, t, :], ones_col16[:],
                                     start=False, stop=sp, skip_group_check=True)

                cs = opool.tile([K, D], f32, tag="cs")
                nc.vector.tensor_scalar_mul(cs[:], cw_sb[:], pe[:, 256:257])
                ef = opool.tile([K, D], f32, tag="ef")
                nc.vector.tensor_sub(ef[:], pe[:, 0:256], cs[:])
                nc.sync.dma_start(eout[b], ef[:])

    _split_multi_waits(nc)
    return nc


_NC_CACHE = None


def _run(X, codewords, scale, trace=False, tmpdir=None):
    global _NC_CACHE
    if _NC_CACHE is None:
        _NC_CACHE = _build_module()
    nc = _NC_CACHE
    Xr = np.ascontiguousarray(X.reshape(B, D, N), dtype=np.float32)
    cw = np.ascontiguousarray(codewords, dtype=np.float32)
    s = np.asarray(scale, dtype=np.float32).reshape(-1)
    in_maps = []
    for c in range(NCORES):
        in_maps.append({
            "xin": Xr[c * BPC:(c + 1) * BPC],
            "cw": cw,
            "s_col": np.ascontiguousarray(s.reshape(K, 1)),
            "s_row": np.ascontiguousarray(s.reshape(1, K)),
        })
    kr = run_bass_kernel_spmd(nc, in_maps, list(range(NCORES)),
                              trace=trace, tmpdir=tmpdir)
    out = np.concatenate([r["eout"] for r in kr.results], axis=0)
    return out.astype(np.float32), kr


def kernel(X, codewords, scale):
    out, _ = _run(X, codewords, scale)
    return out



# revision 20
# speedup vs baseline: 1.7163x; 1.7163x over previous
"""VQ codebook soft-assignment encoding kernel for 8 trn2 NeuronCores.

Math (per batch b):
  Xf = X[b].reshape(D, N).T                        # [N, D], N = H*W
  logit[n,k] = scale[k] * (||x_n||^2 - 2<x_n,c_k> + ||c_k||^2)
  A = softmax(logit, axis=k)
  E[b,k,:] = sum_n A[n,k] * (x_n - c_k)            # [K, D]

Sharding: data-parallel over B (4 batches per core), codewords/scale replicated.

v3 device plan (no on-chip X transposes; X staged fp16 in both layouts):
  - host uploads x16 [BPC, D, N] and xt16 [BPC, XTJ, 128, CPJ, D] (pure layout
    permutation + fp16 round, same values the v1 SWDGE cast-load produced);
    x16 loads are chunked so the logits pass starts early
  - logits in [k, n] layout:  PSUM = G^T X + S^T X^2  where G[d,k] = -2 s_k
    c[k,d], S[d,k] = s_k; U = exp(PSUM + bias_k), bias_k = s_k ||c_k||^2
    (ACT per-partition bias)
  - U^T via one DMA xbar transpose per batch ([32, N] -> [128, 72, 32])
  - normalize on DVE: den = reduce_k, A = U * (1/den)
  - E-matmul on PE: pe[32, 257] += A_t^T @ XT_t (+ ones col) over 72 n-chunks
  - E = pe[:, :256] - pe[:, 256] * c  (DVE), DMA out fp32
"""
import numpy as np
from contextlib import ExitStack

import concourse.bass as bass
import concourse.mybir as mybir
import concourse.tile as tile
from concourse.tile import ScopedClock
from concourse.bass_utils import run_bass_kernel_spmd
from concourse import masks

dt = mybir.dt

B, D, K, H, W = 32, 256, 32, 96, 96
N = H * W                 # 9216
NCORES = 8
BPC = B // NCORES         # 4 batches per core
TN = 512                  # n-tile for logits pass
NT = N // TN              # 18
NCHUNK = N // 128         # 72 e-matmul chunks
SQG = 3                   # n-tiles per square group
XTJ = 4                   # xt load tiles per batch
CPJ = NCHUNK // XTJ       # chunks per xt tile (18)
XLC = 3                   # x16 load chunks per d-half


def _patch_tile_drain():
    """This toolchain's walrus allows only one sync-wait per instruction.
    Split the tail drain's waits across chained drains."""
    if getattr(tile.TileContext, "_drain_patched", False):
        return

    def _drain_and_barrier_split(self, tick_clock, wait_clock):
        nc = self.nc
        drain_inst = nc.sync.drain()
        wait_clock.add_sem_waits(
            drain_inst.ins, ScopedClock({None: tick_clock.global_clock})
        )
        si = drain_inst.ins.sync_info
        if si is not None and si.on_wait and len(si.on_wait) > 1:
            extra = list(si.on_wait[1:])
            del si.on_wait[1:]
            for w in extra:
                d = nc.sync.drain()
                dsi = d.ins.sync_info
                if dsi is None:
                    d.ins.sync_info = mybir.SyncInfo(on_wait=[w], on_update=[])
                else:
                    dsi.on_wait.append(w)
        nc.all_engine_barrier()
        assert self.sems is not None
        popped = nc._tile_sem_poison_stack.pop()
        assert popped is self._sem_poison
        nc.clear_and_free_semaphores(list(self.sems.allocated().values()))
        nc.all_engine_barrier()

    tile.TileContext._drain_and_barrier = _drain_and_barrier_split
    tile.TileContext._drain_patched = True


def _split_multi_waits(nc):
    """Hoist extra sem-waits onto standalone event-sem instructions."""
    n_split = 0
    for f in nc.m.functions:
        for bb in f.blocks:
            new_list = []
            for inst in bb.instructions:
                si = inst.sync_info
                if si is not None and si.on_wait is not None and len(si.on_wait) > 1:
                    extra = list(si.on_wait[:-1])
                    keep = [si.on_wait[-1]]
                    for w in extra:
                        ev = mybir.InstEventSemaphore(
                            name=f"{inst.name}-wsplit{n_split}",
                            ins=[], outs=[],
                            sync_info=mybir.SyncInfo(on_wait=[w], on_update=[]),
                        )
                        ev.engine = inst.engine
                        nc.register_instruction(ev)
                        new_list.append(ev)
                        n_split += 1
                    del si.on_wait[:]
                    si.on_wait.extend(keep)
                new_list.append(inst)
            bb.instructions[:] = new_list
    return n_split


def _build_module():
    _patch_tile_drain()
    nc = bass.Bass()
    x16 = nc.declare_dram_parameter("x16", [BPC, D, N], dt.float16, isOutput=False)
    xt16 = nc.declare_dram_parameter(
        "xt16", [BPC, XTJ, 128, CPJ * D], dt.float16, isOutput=False)
    cw = nc.declare_dram_parameter("cw", [K, D], dt.float32, isOutput=False)
    s_col = nc.declare_dram_parameter("s_col", [K, 1], dt.float32, isOutput=False)
    s_row = nc.declare_dram_parameter("s_row", [1, K], dt.float32, isOutput=False)
    eout = nc.declare_dram_parameter("eout", [BPC, K, D], dt.float32, isOutput=True)

    f16, f32 = dt.float16, dt.float32
    AX = mybir.AxisListType.X
    EXP = mybir.ActivationFunctionType.Exp

    with tile.TileContext(nc) as tc:
        with ExitStack() as ctx:
            singles = ctx.enter_context(tc.tile_pool(name="singles", bufs=1))
            psprep = ctx.enter_context(tc.tile_pool(name="psprep", bufs=1, space="PSUM"))

            # ---- one-time prep from codewords/scale ----
            cw_sb = singles.tile([K, D], f32)
            nc.sync.dma_start(cw_sb[:], cw[:])
            scol_sb = singles.tile([K, 1], f32)
            nc.sync.dma_start(scol_sb[:], s_col[:])
            srow_sb = singles.tile([1, K], f32)
            nc.sync.dma_start(srow_sb[:], s_row[:])

            # G16 [128, 2, K]: G[p, c, k] = -2 s_k c[k, c*128+p]
            w1 = singles.tile([K, D], f32)
            nc.vector.tensor_scalar_mul(w1[:], cw_sb[:], scol_sb[:])
            w2 = singles.tile([K, D], f32)
            nc.vector.tensor_scalar_mul(w2[:], w1[:], -2.0)
            w16 = singles.tile([K, D], f16)
            nc.vector.tensor_copy(w16[:], w2[:])
            g16 = singles.tile([128, 2 * K], f16)
            nc.sync.dma_start_transpose(
                g16[:].rearrange("p (c k) -> p c k", k=K), w16[:]
            )

            # S16 [128, K]: every row = s_k (fp16)
            ones_row16 = singles.tile([1, 128], f16)
            nc.vector.memset(ones_row16[:], 1.0)
            srow16 = singles.tile([1, K], f16)
            nc.vector.tensor_copy(srow16[:], srow_sb[:])
            ps_s = psprep.tile([128, K], f32)
            nc.tensor.matmul(ps_s[:], ones_row16[:], srow16[:], start=True, stop=True,
                             skip_group_check=True)
            s16 = singles.tile([128, K], f16)
            nc.vector.tensor_copy(s16[:], ps_s[:])

            # bias [K, 1] = s_k * ||c_k||^2
            csq = singles.tile([K, D], f32)
            nc.vector.tensor_mul(csq[:], cw_sb[:], cw_sb[:])
            sqc = singles.tile([K, 1], f32)
            nc.vector.reduce_sum(
                sqc[:].rearrange("k (o p) -> k o p", o=1),
                csq[:].rearrange("k (o d) -> k o d", o=1), axis=AX)
            bias = singles.tile([K, 1], f32)
            nc.vector.tensor_mul(bias[:], sqc[:], scol_sb[:])

            ones_col16 = singles.tile([128, 1], f16)
            nc.vector.memset(ones_col16[:], 1.0)

            # ---- per-batch pools ----
            xpool = ctx.enter_context(tc.tile_pool(name="x16p", bufs=2))
            xtpool = ctx.enter_context(tc.tile_pool(name="xt", bufs=6))
            sqpool = ctx.enter_context(tc.tile_pool(name="xsq", bufs=2))
            upool = ctx.enter_context(tc.tile_pool(name="u16", bufs=2))
            utpool = ctx.enter_context(tc.tile_pool(name="ut", bufs=2))
            apool = ctx.enter_context(tc.tile_pool(name="a16", bufs=2))
            npool = ctx.enter_context(tc.tile_pool(name="nrm", bufs=3))
            opool = ctx.enter_context(tc.tile_pool(name="out", bufs=2))
            psl = ctx.enter_context(tc.tile_pool(name="psl", bufs=3, space="PSUM"))
            pse = ctx.enter_context(tc.tile_pool(name="pse", bufs=2, space="PSUM"))

            for b in range(BPC):
                # chunked loads so the first logits tiles can start early;
                # interleave the two d-halves on the queue
                x0 = xpool.tile([128, N], f16, tag="x0")
                x1 = xpool.tile([128, N], f16, tag="x1")
                NL = N // XLC
                for l in range(XLC):
                    ls = bass.ts(l, NL)
                    nc.gpsimd.dma_start(x0[:, ls], x16[b, 0:128, ls])
                    nc.gpsimd.dma_start(x1[:, ls], x16[b, 128:256, ls])

                # XT tiles [128, CPJ*256], fully contiguous per partition
                xts = []
                for j in range(XTJ):
                    xtt = xtpool.tile([128, CPJ * D], f16, tag="xt")
                    nc.sync.dma_start(xtt[:], xt16[b, j])
                    xts.append(xtt[:].rearrange("p (c d) -> p c d", d=D))

                u16 = upool.tile([K, N], f16)
                for g in range(NT // SQG):
                    xsq = sqpool.tile([128, 2 * SQG * TN], f16, tag="xsq")
                    xsq3 = xsq[:].rearrange("p (c m) -> p c m", c=2)
                    sl = bass.ts(g, SQG * TN)
                    nc.vector.tensor_mul(xsq3[:, 0, :], x0[:, sl], x0[:, sl])
                    nc.vector.tensor_mul(xsq3[:, 1, :], x1[:, sl], x1[:, sl])
                    for j in range(SQG):
                        i = g * SQG + j
                        pl = psl.tile([K, TN], f32)
                        xs = bass.ts(i, TN)
                        js = bass.ts(j, TN)
                        g16v = g16[:].rearrange("p (c k) -> p c k", k=K)
                        nc.tensor.matmul(pl[:], g16v[:, 0, :], x0[:, xs],
                                         start=True, stop=False, skip_group_check=True)
                        nc.tensor.matmul(pl[:], g16v[:, 1, :], x1[:, xs],
                                         start=False, stop=False, skip_group_check=True)
                        nc.tensor.matmul(pl[:], s16[:], xsq3[:, 0, js],
                                         start=False, stop=False, skip_group_check=True)
                        nc.tensor.matmul(pl[:], s16[:], xsq3[:, 1, js],
                                         start=False, stop=True, skip_group_check=True)
                        nc.scalar.activation(u16[:, xs], pl[:], EXP,
                                             bias=bias[:], scale=1.0)

                # U^T via DMA xbar transpose: ut3[m, t, k] = U[k, 128t+m]
                ut = utpool.tile([128, NCHUNK * K], f16)
                ut3 = ut[:].rearrange("p (t k) -> p t k", k=K)
                nc.sync.dma_start_transpose(ut3, u16[:])

                a16 = apool.tile([128, NCHUNK * K], f16)
                a3 = a16[:].rearrange("p (t k) -> p t k", k=K)
                NG = 36
                for g in range(NCHUNK // NG):
                    den = npool.tile([128, NG], f32, tag="den")
                    den3 = den[:].rearrange("p (t o) -> p t o", o=1)
                    sl3 = ut3[:, g * NG:(g + 1) * NG, :]
                    nc.vector.reduce_sum(den3, sl3, axis=AX)
                    rec = npool.tile([128, NG], f32, tag="rec")
                    nc.vector.reciprocal(rec[:], den[:])
                    recb = rec[:].rearrange("p (t o) -> p t o", o=1).broadcast_to(
                        (128, NG, K))
                    nc.vector.tensor_mul(a3[:, g * NG:(g + 1) * NG, :], sl3, recb)

                pe = pse.tile([K, 257], f32)
                for t in range(NCHUNK):
                    sp = (t == NCHUNK - 1)
                    nc.tensor.matmul(pe[:, 0:256], a3[:, t, :],
                                     xts[t // CPJ][:, t % CPJ, :],
                                     start=(t == 0), stop=sp,
                                     skip_group_check=True)
                    nc.tensor.matmul(pe[:, 256:257], a3[:, t, :], ones_col16[:],
                                     start=False, stop=sp, skip_group_check=True)

                cs = opool.tile([K, D], f32, tag="cs")
                nc.vector.tensor_scalar_mul(cs[:], cw_sb[:], pe[:, 256:257])
                ef = opool.tile([K, D], f32, tag="ef")
                nc.vector.tensor_sub(ef[:], pe[:, 0:256], cs[:])
                nc.sync.dma_start(eout[b], ef[:])

    _split_multi_waits(nc)
    return nc


_NC_CACHE = None


def _run(X, codewords, scale, trace=False, tmpdir=None):
    global _NC_CACHE
    if _NC_CACHE is None:
        _NC_CACHE = _build_module()
    nc = _NC_CACHE
    Xr = np.asarray(X, dtype=np.float32).reshape(B, D, N)
    x16 = Xr.astype(np.float16)
    # [B, XTJ, 128, CPJ*D]: per-partition-contiguous XT tiles
    xt16 = np.ascontiguousarray(
        x16.transpose(0, 2, 1).reshape(B, XTJ, CPJ, 128, D).transpose(0, 1, 3, 2, 4)
    ).reshape(B, XTJ, 128, CPJ * D)
    cw = np.ascontiguousarray(codewords, dtype=np.float32)
    s = np.asarray(scale, dtype=np.float32).reshape(-1)
    in_maps = []
    for c in range(NCORES):
        sl = slice(c * BPC, (c + 1) * BPC)
        in_maps.append({
            "x16": x16[sl],
            "xt16": xt16[sl],
            "cw": cw,
            "s_col": np.ascontiguousarray(s.reshape(K, 1)),
            "s_row": np.ascontiguousarray(s.reshape(1, K)),
        })
    kr = run_bass_kernel_spmd(nc, in_maps, list(range(NCORES)),
                              trace=trace, tmpdir=tmpdir)
    out = np.concatenate([r["eout"] for r in kr.results], axis=0)
    return out.astype(np.float32), kr


def kernel(X, codewords, scale):
    out, _ = _run(X, codewords, scale)
    return out


# revision 27
# speedup vs baseline: 1.9332x; 1.1263x over previous
"""VQ codebook soft-assignment encoding kernel for 8 trn2 NeuronCores.

Math (per batch b):
  Xf = X[b].reshape(D, N).T                        # [N, D], N = H*W
  logit[n,k] = scale[k] * (||x_n||^2 - 2<x_n,c_k> + ||c_k||^2)
  A = softmax(logit, axis=k)
  E[b,k,:] = sum_n A[n,k] * (x_n - c_k)            # [K, D]

Sharding: data-parallel over B (4 batches per core), codewords/scale replicated.

v3 device plan (no on-chip X transposes; X staged fp16 in both layouts):
  - host uploads x16 [BPC, D, N] and xt16 [BPC, XTJ, 128, CPJ, D] (pure layout
    permutation + fp16 round, same values the v1 SWDGE cast-load produced);
    x16 loads are chunked so the logits pass starts early
  - logits in [k, n] layout:  PSUM = G^T X + S^T X^2  where G[d,k] = -2 s_k
    c[k,d], S[d,k] = s_k; U = exp(PSUM + bias_k), bias_k = s_k ||c_k||^2
    (ACT per-partition bias)
  - U^T via one DMA xbar transpose per batch ([32, N] -> [128, 72, 32])
  - normalize on DVE: den = reduce_k, A = U * (1/den)
  - E-matmul on PE: pe[32, 257] += A_t^T @ XT_t (+ ones col) over 72 n-chunks
  - E = pe[:, :256] - pe[:, 256] * c  (DVE), DMA out fp32
"""
import numpy as np
from contextlib import ExitStack

import concourse.bass as bass
import concourse.mybir as mybir
import concourse.tile as tile
from concourse.tile import ScopedClock
from concourse.bass_utils import run_bass_kernel_spmd
from concourse import masks

dt = mybir.dt

B, D, K, H, W = 32, 256, 32, 96, 96
N = H * W                 # 9216
NCORES = 8
BPC = B // NCORES         # 4 batches per core
TN = 512                  # n-tile for logits pass
NT = N // TN              # 18
NCHUNK = N // 128         # 72 e-matmul chunks
SQG = 3                   # n-tiles per square group
XTJ = 4                   # xt load tiles per batch
CPJ = NCHUNK // XTJ       # chunks per xt tile (18)
XLC = 3                   # x16 load chunks per d-half
XTW = D + 1               # xt row width: 256 d cols + baked-in ones col


def _patch_tile_drain():
    """This toolchain's walrus allows only one sync-wait per instruction.
    Split the tail drain's waits across chained drains."""
    if getattr(tile.TileContext, "_drain_patched", False):
        return

    def _drain_and_barrier_split(self, tick_clock, wait_clock):
        nc = self.nc
        drain_inst = nc.sync.drain()
        wait_clock.add_sem_waits(
            drain_inst.ins, ScopedClock({None: tick_clock.global_clock})
        )
        si = drain_inst.ins.sync_info
        if si is not None and si.on_wait and len(si.on_wait) > 1:
            extra = list(si.on_wait[1:])
            del si.on_wait[1:]
            for w in extra:
                d = nc.sync.drain()
                dsi = d.ins.sync_info
                if dsi is None:
                    d.ins.sync_info = mybir.SyncInfo(on_wait=[w], on_update=[])
                else:
                    dsi.on_wait.append(w)
        nc.all_engine_barrier()
        assert self.sems is not None
        popped = nc._tile_sem_poison_stack.pop()
        assert popped is self._sem_poison
        nc.clear_and_free_semaphores(list(self.sems.allocated().values()))
        nc.all_engine_barrier()

    tile.TileContext._drain_and_barrier = _drain_and_barrier_split
    tile.TileContext._drain_patched = True


def _split_multi_waits(nc):
    """Hoist extra sem-waits onto standalone event-sem instructions."""
    n_split = 0
    for f in nc.m.functions:
        for bb in f.blocks:
            new_list = []
            for inst in bb.instructions:
                si = inst.sync_info
                if si is not None and si.on_wait is not None and len(si.on_wait) > 1:
                    extra = list(si.on_wait[:-1])
                    keep = [si.on_wait[-1]]
                    for w in extra:
                        ev = mybir.InstEventSemaphore(
                            name=f"{inst.name}-wsplit{n_split}",
                            ins=[], outs=[],
                            sync_info=mybir.SyncInfo(on_wait=[w], on_update=[]),
                        )
                        ev.engine = inst.engine
                        nc.register_instruction(ev)
                        new_list.append(ev)
                        n_split += 1
                    del si.on_wait[:]
                    si.on_wait.extend(keep)
                new_list.append(inst)
            bb.instructions[:] = new_list
    return n_split


def _build_module():
    _patch_tile_drain()
    nc = bass.Bass()
    x16 = nc.declare_dram_parameter("x16", [BPC, D, N], dt.float16, isOutput=False)
    xt16 = nc.declare_dram_parameter(
        "xt16", [BPC, XTJ, 128, CPJ * XTW], dt.float16, isOutput=False)
    cw = nc.declare_dram_parameter("cw", [K, D], dt.float32, isOutput=False)
    s_col = nc.declare_dram_parameter("s_col", [K, 1], dt.float32, isOutput=False)
    s_row = nc.declare_dram_parameter("s_row", [1, K], dt.float32, isOutput=False)
    eout = nc.declare_dram_parameter("eout", [BPC, K, D], dt.float32, isOutput=True)

    f16, f32 = dt.float16, dt.float32
    AX = mybir.AxisListType.X
    EXP = mybir.ActivationFunctionType.Exp

    with tile.TileContext(nc) as tc:
        with ExitStack() as ctx:
            singles = ctx.enter_context(tc.tile_pool(name="singles", bufs=1))
            psprep = ctx.enter_context(tc.tile_pool(name="psprep", bufs=1, space="PSUM"))

            # ---- one-time prep from codewords/scale ----
            cw_sb = singles.tile([K, D], f32)
            nc.sync.dma_start(cw_sb[:], cw[:])
            scol_sb = singles.tile([K, 1], f32)
            nc.sync.dma_start(scol_sb[:], s_col[:])
            srow_sb = singles.tile([1, K], f32)
            nc.sync.dma_start(srow_sb[:], s_row[:])

            # G16 [128, 2, K]: G[p, c, k] = -2 s_k c[k, c*128+p]
            w1 = singles.tile([K, D], f32)
            nc.vector.tensor_scalar_mul(w1[:], cw_sb[:], scol_sb[:])
            w2 = singles.tile([K, D], f32)
            nc.vector.tensor_scalar_mul(w2[:], w1[:], -2.0)
            w16 = singles.tile([K, D], f16)
            nc.vector.tensor_copy(w16[:], w2[:])
            g16 = singles.tile([128, 2 * K], f16)
            nc.sync.dma_start_transpose(
                g16[:].rearrange("p (c k) -> p c k", k=K), w16[:]
            )

            # S16 [128, K]: every row = s_k (fp16)
            ones_row16 = singles.tile([1, 128], f16)
            nc.vector.memset(ones_row16[:], 1.0)
            srow16 = singles.tile([1, K], f16)
            nc.vector.tensor_copy(srow16[:], srow_sb[:])
            ps_s = psprep.tile([128, K], f32)
            nc.tensor.matmul(ps_s[:], ones_row16[:], srow16[:], start=True, stop=True,
                             skip_group_check=True)
            s16 = singles.tile([128, K], f16)
            nc.vector.tensor_copy(s16[:], ps_s[:])

            # bias [K, 1] = s_k * ||c_k||^2
            csq = singles.tile([K, D], f32)
            nc.vector.tensor_mul(csq[:], cw_sb[:], cw_sb[:])
            sqc = singles.tile([K, 1], f32)
            nc.vector.reduce_sum(
                sqc[:].rearrange("k (o p) -> k o p", o=1),
                csq[:].rearrange("k (o d) -> k o d", o=1), axis=AX)
            bias = singles.tile([K, 1], f32)
            nc.vector.tensor_mul(bias[:], sqc[:], scol_sb[:])

            ones_col16 = singles.tile([128, 1], f16)
            nc.vector.memset(ones_col16[:], 1.0)

            # ---- per-batch pools ----
            xpool = ctx.enter_context(tc.tile_pool(name="x16p", bufs=2))
            xtpool = ctx.enter_context(tc.tile_pool(name="xt", bufs=8))
            sqpool = ctx.enter_context(tc.tile_pool(name="xsq", bufs=2))
            upool = ctx.enter_context(tc.tile_pool(name="u16", bufs=1))
            utpool = ctx.enter_context(tc.tile_pool(name="ut", bufs=2))
            apool = ctx.enter_context(tc.tile_pool(name="a16", bufs=2))
            npool = ctx.enter_context(tc.tile_pool(name="nrm", bufs=3))
            opool = ctx.enter_context(tc.tile_pool(name="out", bufs=2))
            psl = ctx.enter_context(tc.tile_pool(name="psl", bufs=3, space="PSUM"))
            pse = ctx.enter_context(tc.tile_pool(name="pse", bufs=2, space="PSUM"))

            def emit_norm_and_e(b, ut3, xts):
                """Normalize + E-matmul for batch b, deferred until after the
                next batch's xsq/logits so neither DVE nor PE ever stalls."""
                a16 = apool.tile([128, NCHUNK * K], f16)
                a3 = a16[:].rearrange("p (t k) -> p t k", k=K)
                NG = 36
                for g in range(NCHUNK // NG):
                    den = npool.tile([128, NG], f32, tag="den")
                    den3 = den[:].rearrange("p (t o) -> p t o", o=1)
                    sl3 = ut3[:, g * NG:(g + 1) * NG, :]
                    nc.vector.reduce_sum(den3, sl3, axis=AX)
                    rec = npool.tile([128, NG], f32, tag="rec")
                    nc.vector.reciprocal(rec[:], den[:])
                    recb = rec[:].rearrange("p (t o) -> p t o", o=1).broadcast_to(
                        (128, NG, K))
                    nc.vector.tensor_mul(a3[:, g * NG:(g + 1) * NG, :], sl3, recb)

                pe = pse.tile([K, 257], f32)
                for t in range(NCHUNK):
                    nc.tensor.matmul(pe[:], a3[:, t, :],
                                     xts[t // CPJ][:, t % CPJ, :],
                                     start=(t == 0), stop=(t == NCHUNK - 1),
                                     skip_group_check=True)
                cs = opool.tile([K, D], f32, tag="cs")
                nc.vector.tensor_scalar_mul(cs[:], cw_sb[:], pe[:, 256:257])
                ef = opool.tile([K, D], f32, tag="ef")
                nc.vector.tensor_sub(ef[:], pe[:, 0:256], cs[:])
                nc.sync.dma_start(eout[b], ef[:])

            pending = None
            for b in range(BPC):
                # chunked loads so the first logits tiles can start early;
                # interleave the two d-halves on the queue
                x0 = xpool.tile([128, N], f16, tag="x0")
                x1 = xpool.tile([128, N], f16, tag="x1")
                NL = N // XLC
                for l in range(XLC):
                    ls = bass.ts(l, NL)
                    nc.gpsimd.dma_start(x0[:, ls], x16[b, 0:128, ls])
                    nc.gpsimd.dma_start(x1[:, ls], x16[b, 128:256, ls])

                # XT tiles [128, CPJ*257] (ones col baked in), contiguous
                xts = []
                for j in range(XTJ):
                    xtt = xtpool.tile([128, CPJ * XTW], f16, tag="xt")
                    nc.sync.dma_start(xtt[:], xt16[b, j])
                    xts.append(xtt[:].rearrange("p (c d) -> p c d", d=XTW))

                u16 = upool.tile([K, N], f16)
                for g in range(NT // SQG):
                    xsq = sqpool.tile([128, 2 * SQG * TN], f16, tag="xsq")
                    xsq3 = xsq[:].rearrange("p (c m) -> p c m", c=2)
                    sl = bass.ts(g, SQG * TN)
                    nc.vector.tensor_mul(xsq3[:, 0, :], x0[:, sl], x0[:, sl])
                    nc.vector.tensor_mul(xsq3[:, 1, :], x1[:, sl], x1[:, sl])
                    for j in range(SQG):
                        i = g * SQG + j
                        pl = psl.tile([K, TN], f32)
                        xs = bass.ts(i, TN)
                        js = bass.ts(j, TN)
                        g16v = g16[:].rearrange("p (c k) -> p c k", k=K)
                        nc.tensor.matmul(pl[:], g16v[:, 0, :], x0[:, xs],
                                         start=True, stop=False, skip_group_check=True)
                        nc.tensor.matmul(pl[:], g16v[:, 1, :], x1[:, xs],
                                         start=False, stop=False, skip_group_check=True)
                        nc.tensor.matmul(pl[:], s16[:], xsq3[:, 0, js],
                                         start=False, stop=False, skip_group_check=True)
                        nc.tensor.matmul(pl[:], s16[:], xsq3[:, 1, js],
                                         start=False, stop=True, skip_group_check=True)
                        nc.scalar.activation(u16[:, xs], pl[:], EXP,
                                             bias=bias[:], scale=1.0)

                # U^T via DMA xbar transpose: ut3[m, t, k] = U[k, 128t+m]
                ut = utpool.tile([128, NCHUNK * K], f16)
                ut3 = ut[:].rearrange("p (t k) -> p t k", k=K)
                nc.sync.dma_start_transpose(ut3, u16[:])

                if pending is not None:
                    emit_norm_and_e(*pending)
                pending = (b, ut3, xts)

            emit_norm_and_e(*pending)

    _split_multi_waits(nc)
    return nc


_NC_CACHE = None


def _run(X, codewords, scale, trace=False, tmpdir=None):
    global _NC_CACHE
    if _NC_CACHE is None:
        _NC_CACHE = _build_module()
    nc = _NC_CACHE
    Xr = np.asarray(X, dtype=np.float32).reshape(B, D, N)
    x16 = Xr.astype(np.float16)
    # [B, XTJ, 128, CPJ, 257]: per-partition-contiguous XT tiles with a
    # baked-in ones column (col 256) so the E-matmul accumulates S_k too
    xt16 = np.empty((B, XTJ, 128, CPJ, XTW), dtype=np.float16)
    xt16[..., 256] = 1.0
    xt16[..., 0:256] = (
        x16.transpose(0, 2, 1).reshape(B, XTJ, CPJ, 128, D).transpose(0, 1, 3, 2, 4))
    xt16 = xt16.reshape(B, XTJ, 128, CPJ * XTW)
    cw = np.ascontiguousarray(codewords, dtype=np.float32)
    s = np.asarray(scale, dtype=np.float32).reshape(-1)
    in_maps = []
    for c in range(NCORES):
        sl = slice(c * BPC, (c + 1) * BPC)
        in_maps.append({
            "x16": x16[sl],
            "xt16": xt16[sl],
            "cw": cw,
            "s_col": np.ascontiguousarray(s.reshape(K, 1)),
            "s_row": np.ascontiguousarray(s.reshape(1, K)),
        })
    kr = run_bass_kernel_spmd(nc, in_maps, list(range(NCORES)),
                              trace=trace, tmpdir=tmpdir)
    out = np.concatenate([r["eout"] for r in kr.results], axis=0)
    return out.astype(np.float32), kr


def kernel(X, codewords, scale):
    out, _ = _run(X, codewords, scale)
    return out
